# revision 31
# baseline (speedup 1.0000x reference)
"""ColBERT loss kernel for Trainium2, SPMD over 8 NeuronCores.

Problem: q [64,64,128], d_pos/d_neg [64,512,128], mask_pos/neg [64,512] ->
scalar CE loss over maxsim scores [64, 128].

Strategy:
- Shard the 128 docs (64 pos + 64 neg) across 8 cores: 16 docs/core
  (8 pos + 8 neg), replicate q.  Cross-entropy needs full rows, so the
  [64, 16] per-core score slabs are gathered and the tiny [64,128]
  softmax+CE epilogue runs on host.
- Mask folding on host: invalid doc tokens are replaced by that doc's
  token 0 (always valid per setup), so max over all 512 positions ==
  masked max, exactly.  No mask work on device.
- Per core: for each query pair p (two queries -> 128 partitions) and
  doc d: PE matmul sim = qT_p.T @ dT_d -> PSUM [128, 512]; DVE
  reduce-max over t -> maxvals[:, p, d]; then one matmul per 8-pair
  group with a ones-mask splits/sums the two 64-partition halves ->
  scores [16 docs, 2 queries] per pair.
- Near-fp32 accuracy at 2 cycles/column via fp16 hi/lo split of q:
  sim = qh@dh + ql@dh = q@dh accumulated in fp32 PSUM; the only error
  is the dropped d low part (~2^-11 per product; measured 1.4e-5
  relative on the final loss).  Set N_PASSES = 3 to also accumulate
  qh@dl (~2e-7 relative) at ~40% more PE time.
"""

import numpy as np

import concourse.bass as bass
import concourse.mybir as mybir
import concourse.tile as tile
from concourse.bass_utils import run_bass_kernel_spmd
from concourse.vector_clock import ScopedClock

B, SQ, SD, H = 64, 64, 512, 128
NCORES = 8
DOCS_PER_CORE = 16  # 8 pos + 8 neg
NPAIRS = B // 2  # 32 query pairs
PAIR_GROUPS = 4  # 8 pairs per sum-matmul group
RED_BATCH = 2  # docs per DVE reduce


def _patch_tile_drain():
    """walrus rejects >1 sync-wait on a Drain (CTRL) instruction; the
    TileContext tail drain carries one wait per outstanding semaphore.
    Split them across a chain of single-wait drains."""
    if getattr(tile.TileContext, "_drain_patched", False):
        return

    def _drain_and_barrier(self, tick_clock, wait_clock):
        nc = self.nc
        drain_inst = nc.sync.drain()
        wait_clock.add_sem_waits(
            drain_inst.ins, ScopedClock({None: tick_clock.global_clock})
        )
        si = drain_inst.ins.sync_info
        conds = list(si.on_wait) if (si is not None and si.on_wait) else []
        if len(conds) > 1:
            upd = list(si.on_update) if (si is not None and si.on_update) else []
            drain_inst.ins.sync_info = mybir.SyncInfo(on_wait=conds[:1], on_update=upd)
            for c in conds[1:]:
                extra = nc.sync.drain()
                extra.ins.sync_info = mybir.SyncInfo(on_wait=[c], on_update=[])
        nc.all_engine_barrier()
        assert self.sems is not None
        popped = nc._tile_sem_poison_stack.pop()
        assert popped is self._sem_poison
        nc.clear_and_free_semaphores(list(self.sems.allocated().values()))
        nc.all_engine_barrier()

    tile.TileContext._drain_and_barrier = _drain_and_barrier
    tile.TileContext._drain_patched = True


def _split_multi_waits(nc, max_waits=1):
    """This walrus build accepts at most one sync-wait per instruction.
    Hoist extra waits onto same-engine NoOps inserted just before."""
    for f in nc.m.functions:
        for blk in f.blocks:
            new = []
            changed = False
            for inst in blk.instructions:
                si = inst.sync_info
                conds = list(si.on_wait) if (si is not None and si.on_wait) else []
                if len(conds) > max_waits:
                    upd = list(si.on_update) if si.on_update else []
                    for c in conds[:-max_waits]:
                        nop = mybir.InstNoOp(name=f"I-wsplit-{nc.next_id()}")
                        nop.engine = inst.engine
                        nop.sync_info = mybir.SyncInfo(on_wait=[c], on_update=[])
                        new.append(nop)
                    inst.sync_info = mybir.SyncInfo(
                        on_wait=conds[-max_waits:], on_update=upd
                    )
                    changed = True
                new.append(inst)
            if changed:
                blk.instructions = new


def _build_program(loop_repeat=1, probe=None, t_hat=SD, gp_groups=0, n_passes=3, red_batch=RED_BATCH, deep_psum=False):
    """loop_repeat>1 wraps the compute body in a device-side For loop —
    used only for benchmarking (amplifies kernel time above the ~100ms
    axon dispatch noise).  probe: None | "nored" (skip reduces; PE span)
    | "onepass" (1 matmul pass; DVE span).  Probe outputs are garbage.
    t_hat: number of leading token columns to process per doc (compacted
    inputs put all valid tokens first).  gp_groups: how many of the 4
    pair-groups route their max-reduce through ACT-copy + gpsimd.max
    instead of the DVE (load-balancing the reduce across engines)."""
    _patch_tile_drain()
    f32 = mybir.dt.float32
    f16 = mybir.dt.float16
    nc = bass.Bass("TRN2", target_bir_lowering=False, debug=False, num_devices=NCORES)

    qh = nc.dram_tensor("qh", [H, B * SQ], f16, kind="ExternalInput").ap()
    ql = nc.dram_tensor("ql", [H, B * SQ], f16, kind="ExternalInput").ap()
    dh = nc.dram_tensor("dh", [H, DOCS_PER_CORE * SD], f16, kind="ExternalInput").ap()
    dl = nc.dram_tensor("dl", [H, DOCS_PER_CORE * SD], f16, kind="ExternalInput").ap()
    ones2 = nc.dram_tensor("ones2", [H, 2], f32, kind="ExternalInput").ap()
    out = nc.dram_tensor(
        "scores_raw", [H, PAIR_GROUPS * 2], f32, kind="ExternalOutput"
    ).ap()

    import contextlib

    mm_bufs = (4 if deep_psum else 3) if red_batch == 2 else 2
    with tile.TileContext(nc) as tc, contextlib.ExitStack() as es:
        if True:
            const_pool = es.enter_context(tc.tile_pool(name="const", bufs=1))
            mm_pool = es.enter_context(
                tc.tile_pool(name="mm", bufs=mm_bufs, space="PSUM")
            )
            sb_pool = es.enter_context(tc.tile_pool(name="sb", bufs=1))
            stage_pool = es.enter_context(tc.tile_pool(name="stage", bufs=3))
            if red_batch == 2 and not deep_psum:
                sum_pool = es.enter_context(
                    tc.tile_pool(name="sums", bufs=2, space="PSUM")
                )
            else:
                sum_pool = mm_pool
            qh_sb = const_pool.tile([H, B * SQ], f16)
            nc.sync.dma_start(qh_sb[:], qh[:])
            ql_sb = const_pool.tile([H, B * SQ], f16)
            nc.sync.dma_start(ql_sb[:], ql[:])
            dh_sb = const_pool.tile([H, DOCS_PER_CORE * SD], f16)
            nc.sync.dma_start(dh_sb[:], dh[:])
            dl_sb = const_pool.tile([H, DOCS_PER_CORE * SD], f16)
            nc.sync.dma_start(dl_sb[:], dl[:])
            ones2_sb = const_pool.tile([H, 2], f32)
            nc.sync.dma_start(ones2_sb[:], ones2[:])

            maxvals = sb_pool.tile([H, NPAIRS, DOCS_PER_CORE], f32)
            max8 = (
                sb_pool.tile([H, NPAIRS, DOCS_PER_CORE, 8], f32)
                if gp_groups > 0
                else None
            )
            scores_sb = sb_pool.tile([H, PAIR_GROUPS, 2], f32)
            nc.vector.memset(maxvals[:], 0.0)
            nc.vector.memset(scores_sb[:], 0.0)

            def body(_iv=None):
                _emit_body(
                    nc,
                    qh_sb,
                    ql_sb,
                    dh_sb,
                    dl_sb,
                    ones2_sb,
                    maxvals,
                    max8,
                    scores_sb,
                    mm_pool,
                    sum_pool,
                    stage_pool,
                    probe,
                    t_hat,
                    gp_groups,
                    n_passes,
                    red_batch,
                )

            if loop_repeat > 1:
                with tc.For_i(0, loop_repeat, 1):
                    body()
            else:
                body()

            nc.sync.dma_start(out[:], scores_sb[:, :, :])

    _split_multi_waits(nc)
    return nc


def _emit_body(
    nc,
    qh_sb,
    ql_sb,
    dh_sb,
    dl_sb,
    ones2_sb,
    maxvals,
    max8,
    scores_sb,
    mm_pool,
    sum_pool,
    stage_pool,
    probe=None,
    t_hat=SD,
    gp_groups=0,
    n_passes=3,
    red_batch=RED_BATCH,
):
    f32 = mybir.dt.float32
    n_batches = DOCS_PER_CORE // red_batch
    batch_ts = [t_hat] * n_batches if isinstance(t_hat, int) else list(t_hat)
    assert len(batch_ts) == n_batches
    # pairs in groups >= PAIR_GROUPS - gp_groups route reduces through
    # ACT copy + vector.max (top-8) instead of direct DVE reduce
    gp_pair_start = (PAIR_GROUPS - gp_groups) * 8
    for p in range(NPAIRS):
        qslice = slice(p * 128, (p + 1) * 128)
        use_gp = p >= gp_pair_start
        for b in range(n_batches):
            t_b = batch_ts[b]
            ps = mm_pool.tile([H, red_batch, SD], f32, tag="ps")
            # weight-major order: one stationary load serves RED_BATCH docs
            if probe == "onepass":
                passes = [(qh_sb, dh_sb, True, True)]
            elif n_passes == 2:
                passes = [
                    (qh_sb, dh_sb, True, False),
                    (ql_sb, dh_sb, False, True),
                ]
            else:
                passes = [
                    (qh_sb, dh_sb, True, False),
                    (ql_sb, dh_sb, False, False),
                    (qh_sb, dl_sb, False, True),
                ]
            for w_sb, m_sb, is_start, is_stop in passes:
                for j in range(red_batch):
                    d = b * red_batch + j
                    dslice = slice(d * SD, d * SD + t_b)
                    nc.tensor.matmul(
                        ps[:, j, 0:t_b],
                        lhsT=w_sb[:, qslice],
                        rhs=m_sb[:, dslice],
                        start=is_start,
                        stop=is_stop,
                    )
            if probe == "nored":
                continue
            if use_gp:
                stage = stage_pool.tile([H, red_batch, SD], f32)
                nc.scalar.copy(stage[:, :, 0:t_b], ps[:, :, 0:t_b])
                for j in range(red_batch):
                    d = b * red_batch + j
                    nc.vector.max(max8[:, p, d, :], stage[:, j, 0:t_b])
            else:
                nc.vector.tensor_reduce(
                    out=maxvals[:, p, b * red_batch : (b + 1) * red_batch],
                    in_=ps[:, :, 0:t_b],
                    axis=mybir.AxisListType.X,
                    op=mybir.AluOpType.max,
                )

    if probe == "nored":
        return
    for g in range(PAIR_GROUPS):
        sums = sum_pool.tile([H, 2], f32, tag="ps" if sum_pool is mm_pool else "sums")
        if g * 8 >= gp_pair_start:
            lhsT = max8[:, g * 8 : (g + 1) * 8, :, 0]
        else:
            lhsT = maxvals[:, g * 8 : (g + 1) * 8, :]
        nc.tensor.matmul(
            sums[:],
            lhsT=lhsT,
            rhs=ones2_sb[:],
            start=True,
            stop=True,
        )
        nc.vector.tensor_copy(scores_sb[:, g, :], sums[:])


_PROGRAMS = {}


N_PASSES = 2


def _get_program(batch_ts):
    key = (tuple(batch_ts), N_PASSES)
    if key not in _PROGRAMS:
        _PROGRAMS[key] = _build_program(t_hat=tuple(batch_ts), n_passes=N_PASSES)
    return _PROGRAMS[key]


def _host_prep(q, d_pos, d_neg, mask_pos, mask_neg):
    q = np.asarray(q, dtype=np.float32)
    d_pos = np.asarray(d_pos, dtype=np.float32)
    d_neg = np.asarray(d_neg, dtype=np.float32)
    mask_pos = np.asarray(mask_pos)
    mask_neg = np.asarray(mask_neg)

    # Compact: move each doc's valid tokens to the front, pad the tail
    # with copies of token 0 (always valid per setup).  Plain max over
    # the first t_hat columns == masked max, exactly.
    def compact(d, mask):
        out = np.empty_like(d)
        for b in range(d.shape[0]):
            v = d[b, mask[b] != 0]
            out[b, : len(v)] = v
            out[b, len(v) :] = d[b, 0]
        return out

    dp = compact(d_pos, mask_pos)
    dn = compact(d_neg, mask_neg)
    cp = mask_pos.sum(1)
    cn = mask_neg.sum(1)
    # Per core: sort its 16 docs by valid-count descending so doc slots
    # with similar counts share a reduce batch; per-batch T is the max
    # count over the cores' docs in that batch's two slots.
    perms = []
    sorted_counts = np.zeros((NCORES, DOCS_PER_CORE), np.int64)
    for c in range(NCORES):
        counts = np.concatenate([cp[8 * c : 8 * c + 8], cn[8 * c : 8 * c + 8]])
        perm = np.argsort(-counts, kind="stable")
        perms.append(perm)
        sorted_counts[c] = counts[perm]
    n_batches = DOCS_PER_CORE // RED_BATCH
    batch_ts = tuple(
        min(SD, (int(sorted_counts[:, b * RED_BATCH : (b + 1) * RED_BATCH].max()) + 3)
            // 4 * 4)
        for b in range(n_batches)
    )

    def split_hi_lo(x):
        hi = x.astype(np.float16)
        lo = (x - hi.astype(np.float32)).astype(np.float16)
        return hi, lo

    # qT[h, q*SQ + s]
    qT = np.ascontiguousarray(q.transpose(2, 0, 1).reshape(H, B * SQ))
    qh, ql = split_hi_lo(qT)
    # dT[h, doc, t]
    dpT = dp.transpose(2, 0, 1)  # [H, 64, 512]
    dnT = dn.transpose(2, 0, 1)

    ones2 = np.zeros((H, 2), np.float32)
    ones2[:SQ, 0] = 1.0
    ones2[SQ:, 1] = 1.0

    in_maps = []
    for c in range(NCORES):
        dT_c = np.ascontiguousarray(
            np.concatenate(
                [dpT[:, 8 * c : 8 * c + 8, :], dnT[:, 8 * c : 8 * c + 8, :]], axis=1
            )[:, perms[c], :].reshape(H, DOCS_PER_CORE * SD)
        )
        dh_c, dl_c = split_hi_lo(dT_c)
        in_maps.append(
            {
                "qh": qh,
                "ql": ql,
                "dh": dh_c,
                "dl": dl_c,
                "ones2": ones2,
            }
        )
    return in_maps, batch_ts, perms


def _host_epilogue(results, perms):
    # scores_raw rows: partition = pg*16 + d_local; cols: g*2 + j
    # query = 2*(8*g + pg) + j ; doc_local d: 0-7 pos docs 8c+d, 8-15 neg.
    dist = np.zeros((B, 2 * B), np.float32)
    for c in range(NCORES):
        arr = np.asarray(results[c]["scores_raw"])  # [128, 8]
        arr = arr.reshape(8, 16, PAIR_GROUPS, 2)  # [pg, d, g, j]
        s_qd = arr.transpose(2, 0, 3, 1).reshape(B, DOCS_PER_CORE)  # [query, slot]
        inv = np.empty_like(perms[c])
        inv[perms[c]] = np.arange(DOCS_PER_CORE)
        s_qd = s_qd[:, inv]  # [query, original local doc]
        dist[:, 8 * c : 8 * c + 8] = s_qd[:, 0:8]
        dist[:, B + 8 * c : B + 8 * c + 8] = s_qd[:, 8:16]

    d64 = dist.astype(np.float64)
    m = d64.max(axis=1, keepdims=True)
    logz = np.log(np.exp(d64 - m).sum(axis=1)) + m[:, 0]
    lbl = np.arange(B)
    loss = -(d64[lbl, lbl] - logz).mean()
    return np.array(loss, dtype=np.float32)


def kernel(q, d_pos, d_neg, mask_pos, mask_neg):
    in_maps, batch_ts, perms = _host_prep(q, d_pos, d_neg, mask_pos, mask_neg)
    nc = _get_program(batch_ts)
    res = run_bass_kernel_spmd(nc, in_maps, list(range(NCORES)), trace=False)
    return _host_epilogue(res.results, perms)


# revision 32
# speedup vs baseline: 1.0098x; 1.0098x over previous
"""ColBERT loss kernel for Trainium2, SPMD over 8 NeuronCores.

Problem: q [64,64,128], d_pos/d_neg [64,512,128], mask_pos/neg [64,512] ->
scalar CE loss over maxsim scores [64, 128].

Strategy:
- Shard the 128 docs (64 pos + 64 neg) across 8 cores: 16 docs/core
  (8 pos + 8 neg), replicate q.  Cross-entropy needs full rows, so the
  [64, 16] per-core score slabs are gathered and the tiny [64,128]
  softmax+CE epilogue runs on host.
- Mask folding on host: invalid doc tokens are replaced by that doc's
  token 0 (always valid per setup), so max over all 512 positions ==
  masked max, exactly.  No mask work on device.
- Per core: for each query pair p (two queries -> 128 partitions) and
  doc d: PE matmul sim = qT_p.T @ dT_d -> PSUM [128, 512]; DVE
  reduce-max over t -> maxvals[:, p, d]; then one matmul per 8-pair
  group with a ones-mask splits/sums the two 64-partition halves ->
  scores [16 docs, 2 queries] per pair.
- Near-fp32 accuracy at 2 cycles/column via fp16 hi/lo split of q:
  sim = qh@dh + ql@dh = q@dh accumulated in fp32 PSUM; the only error
  is the dropped d low part (~2^-11 per product; measured 1.4e-5
  relative on the final loss).  Set N_PASSES = 3 to also accumulate
  qh@dl (~2e-7 relative) at ~40% more PE time.
"""

import numpy as np

import concourse.bass as bass
import concourse.mybir as mybir
import concourse.tile as tile
from concourse.bass_utils import run_bass_kernel_spmd
from concourse.vector_clock import ScopedClock

B, SQ, SD, H = 64, 64, 512, 128
NCORES = 8
DOCS_PER_CORE = 16  # 8 pos + 8 neg
NPAIRS = B // 2  # 32 query pairs
PAIR_GROUPS = 4  # 8 pairs per sum-matmul group
RED_BATCH = 2  # docs per DVE reduce


def _patch_tile_drain():
    """walrus rejects >1 sync-wait on a Drain (CTRL) instruction; the
    TileContext tail drain carries one wait per outstanding semaphore.
    Split them across a chain of single-wait drains."""
    if getattr(tile.TileContext, "_drain_patched", False):
        return

    def _drain_and_barrier(self, tick_clock, wait_clock):
        nc = self.nc
        drain_inst = nc.sync.drain()
        wait_clock.add_sem_waits(
            drain_inst.ins, ScopedClock({None: tick_clock.global_clock})
        )
        si = drain_inst.ins.sync_info
        conds = list(si.on_wait) if (si is not None and si.on_wait) else []
        if len(conds) > 1:
            upd = list(si.on_update) if (si is not None and si.on_update) else []
            drain_inst.ins.sync_info = mybir.SyncInfo(on_wait=conds[:1], on_update=upd)
            for c in conds[1:]:
                extra = nc.sync.drain()
                extra.ins.sync_info = mybir.SyncInfo(on_wait=[c], on_update=[])
        nc.all_engine_barrier()
        assert self.sems is not None
        popped = nc._tile_sem_poison_stack.pop()
        assert popped is self._sem_poison
        nc.clear_and_free_semaphores(list(self.sems.allocated().values()))
        nc.all_engine_barrier()

    tile.TileContext._drain_and_barrier = _drain_and_barrier
    tile.TileContext._drain_patched = True


def _split_multi_waits(nc, max_waits=1):
    """This walrus build accepts at most one sync-wait per instruction.
    Hoist extra waits onto same-engine NoOps inserted just before."""
    for f in nc.m.functions:
        for blk in f.blocks:
            new = []
            changed = False
            for inst in blk.instructions:
                si = inst.sync_info
                conds = list(si.on_wait) if (si is not None and si.on_wait) else []
                if len(conds) > max_waits:
                    upd = list(si.on_update) if si.on_update else []
                    for c in conds[:-max_waits]:
                        nop = mybir.InstNoOp(name=f"I-wsplit-{nc.next_id()}")
                        nop.engine = inst.engine
                        nop.sync_info = mybir.SyncInfo(on_wait=[c], on_update=[])
                        new.append(nop)
                    inst.sync_info = mybir.SyncInfo(
                        on_wait=conds[-max_waits:], on_update=upd
                    )
                    changed = True
                new.append(inst)
            if changed:
                blk.instructions = new


def _build_program(loop_repeat=1, probe=None, t_hat=SD, gp_groups=0, n_passes=3, red_batch=RED_BATCH, deep_psum=False, stage_every=0):
    """loop_repeat>1 wraps the compute body in a device-side For loop —
    used only for benchmarking (amplifies kernel time above the ~100ms
    axon dispatch noise).  probe: None | "nored" (skip reduces; PE span)
    | "onepass" (1 matmul pass; DVE span).  Probe outputs are garbage.
    t_hat: number of leading token columns to process per doc (compacted
    inputs put all valid tokens first).  gp_groups: how many of the 4
    pair-groups route their max-reduce through ACT-copy + gpsimd.max
    instead of the DVE (load-balancing the reduce across engines)."""
    _patch_tile_drain()
    f32 = mybir.dt.float32
    f16 = mybir.dt.float16
    nc = bass.Bass("TRN2", target_bir_lowering=False, debug=False, num_devices=NCORES)

    qh = nc.dram_tensor("qh", [H, B * SQ], f16, kind="ExternalInput").ap()
    ql = nc.dram_tensor("ql", [H, B * SQ], f16, kind="ExternalInput").ap()
    dh = nc.dram_tensor("dh", [H, DOCS_PER_CORE * SD], f16, kind="ExternalInput").ap()
    dl = nc.dram_tensor("dl", [H, DOCS_PER_CORE * SD], f16, kind="ExternalInput").ap()
    ones2 = nc.dram_tensor("ones2", [H, 2], f32, kind="ExternalInput").ap()
    out = nc.dram_tensor(
        "scores_raw", [H, PAIR_GROUPS * 2], f32, kind="ExternalOutput"
    ).ap()

    import contextlib

    mm_bufs = (4 if deep_psum else 3) if red_batch == 2 else 2
    with tile.TileContext(nc) as tc, contextlib.ExitStack() as es:
        if True:
            const_pool = es.enter_context(tc.tile_pool(name="const", bufs=1))
            mm_pool = es.enter_context(
                tc.tile_pool(name="mm", bufs=mm_bufs, space="PSUM")
            )
            sb_pool = es.enter_context(tc.tile_pool(name="sb", bufs=1))
            stage_pool = es.enter_context(tc.tile_pool(name="stage", bufs=3))
            if red_batch == 2 and not deep_psum:
                sum_pool = es.enter_context(
                    tc.tile_pool(name="sums", bufs=2, space="PSUM")
                )
            else:
                sum_pool = mm_pool
            qh_sb = const_pool.tile([H, B * SQ], f16)
            nc.sync.dma_start(qh_sb[:], qh[:])
            ql_sb = const_pool.tile([H, B * SQ], f16)
            nc.sync.dma_start(ql_sb[:], ql[:])
            dh_sb = const_pool.tile([H, DOCS_PER_CORE * SD], f16)
            nc.sync.dma_start(dh_sb[:], dh[:])
            dl_sb = const_pool.tile([H, DOCS_PER_CORE * SD], f16)
            nc.sync.dma_start(dl_sb[:], dl[:])
            ones2_sb = const_pool.tile([H, 2], f32)
            nc.sync.dma_start(ones2_sb[:], ones2[:])

            maxvals = sb_pool.tile([H, NPAIRS, DOCS_PER_CORE], f32)
            max8 = (
                sb_pool.tile([H, NPAIRS, DOCS_PER_CORE, 8], f32)
                if gp_groups > 0
                else None
            )
            scores_sb = sb_pool.tile([H, PAIR_GROUPS, 2], f32)
            nc.vector.memset(maxvals[:], 0.0)
            nc.vector.memset(scores_sb[:], 0.0)

            def body(_iv=None):
                _emit_body(
                    nc,
                    qh_sb,
                    ql_sb,
                    dh_sb,
                    dl_sb,
                    ones2_sb,
                    maxvals,
                    max8,
                    scores_sb,
                    mm_pool,
                    sum_pool,
                    stage_pool,
                    probe,
                    t_hat,
                    gp_groups,
                    n_passes,
                    red_batch,
                    stage_every,
                )

            if loop_repeat > 1:
                with tc.For_i(0, loop_repeat, 1):
                    body()
            else:
                body()

            nc.sync.dma_start(out[:], scores_sb[:, :, :])

    _split_multi_waits(nc)
    return nc


def _emit_body(
    nc,
    qh_sb,
    ql_sb,
    dh_sb,
    dl_sb,
    ones2_sb,
    maxvals,
    max8,
    scores_sb,
    mm_pool,
    sum_pool,
    stage_pool,
    probe=None,
    t_hat=SD,
    gp_groups=0,
    n_passes=3,
    red_batch=RED_BATCH,
    stage_every=0,
):
    f32 = mybir.dt.float32
    n_batches = DOCS_PER_CORE // red_batch
    batch_ts = [t_hat] * n_batches if isinstance(t_hat, int) else list(t_hat)
    assert len(batch_ts) == n_batches
    # pairs in groups >= PAIR_GROUPS - gp_groups route reduces through
    # ACT copy + vector.max (top-8) instead of direct DVE reduce
    gp_pair_start = (PAIR_GROUPS - gp_groups) * 8
    for p in range(NPAIRS):
        qslice = slice(p * 128, (p + 1) * 128)
        use_gp = p >= gp_pair_start
        for b in range(n_batches):
            t_b = batch_ts[b]
            ps = mm_pool.tile([H, red_batch, SD], f32, tag="ps")
            # weight-major order: one stationary load serves RED_BATCH docs
            if probe == "onepass" or n_passes == 1:
                passes = [(qh_sb, dh_sb, True, True)]
            elif n_passes == 2:
                passes = [
                    (qh_sb, dh_sb, True, False),
                    (ql_sb, dh_sb, False, True),
                ]
            else:
                passes = [
                    (qh_sb, dh_sb, True, False),
                    (ql_sb, dh_sb, False, False),
                    (qh_sb, dl_sb, False, True),
                ]
            for w_sb, m_sb, is_start, is_stop in passes:
                for j in range(red_batch):
                    d = b * red_batch + j
                    dslice = slice(d * SD, d * SD + t_b)
                    nc.tensor.matmul(
                        ps[:, j, 0:t_b],
                        lhsT=w_sb[:, qslice],
                        rhs=m_sb[:, dslice],
                        start=is_start,
                        stop=is_stop,
                    )
            if probe == "nored":
                continue
            if use_gp:
                stage = stage_pool.tile([H, red_batch, SD], f32)
                nc.scalar.copy(stage[:, :, 0:t_b], ps[:, :, 0:t_b])
                for j in range(red_batch):
                    d = b * red_batch + j
                    nc.vector.max(max8[:, p, d, :], stage[:, j, 0:t_b])
            elif stage_every > 0 and (p * n_batches + b) % stage_every == 0:
                # fp16 staging path: ACT casts PSUM->SBUF fp16, DVE folds
                # halves with a 2x-mode tensor_tensor max, then a short
                # 1x reduce.  Costs ~2^-11 rounding on the winning maxima.
                f16 = mybir.dt.float16
                t2 = t_b // 2
                stage16 = stage_pool.tile([H, red_batch, SD], f16, tag="st16")
                nc.scalar.copy(stage16[:, :, 0:t_b], ps[:, :, 0:t_b])
                fold = stage_pool.tile([H, red_batch, SD // 2], f16, tag="fold")
                nc.vector.tensor_tensor(
                    fold[:, :, 0:t2],
                    stage16[:, :, 0:t2],
                    stage16[:, :, t2:t_b],
                    mybir.AluOpType.max,
                )
                nc.vector.tensor_reduce(
                    out=maxvals[:, p, b * red_batch : (b + 1) * red_batch],
                    in_=fold[:, :, 0:t2],
                    axis=mybir.AxisListType.X,
                    op=mybir.AluOpType.max,
                )
            else:
                nc.vector.tensor_reduce(
                    out=maxvals[:, p, b * red_batch : (b + 1) * red_batch],
                    in_=ps[:, :, 0:t_b],
                    axis=mybir.AxisListType.X,
                    op=mybir.AluOpType.max,
                )

    if probe == "nored":
        return
    for g in range(PAIR_GROUPS):
        sums = sum_pool.tile([H, 2], f32, tag="ps" if sum_pool is mm_pool else "sums")
        if g * 8 >= gp_pair_start:
            lhsT = max8[:, g * 8 : (g + 1) * 8, :, 0]
        else:
            lhsT = maxvals[:, g * 8 : (g + 1) * 8, :]
        nc.tensor.matmul(
            sums[:],
            lhsT=lhsT,
            rhs=ones2_sb[:],
            start=True,
            stop=True,
        )
        nc.vector.tensor_copy(scores_sb[:, g, :], sums[:])


_PROGRAMS = {}


N_PASSES = 2


def _get_program(batch_ts):
    key = (tuple(batch_ts), N_PASSES)
    if key not in _PROGRAMS:
        _PROGRAMS[key] = _build_program(t_hat=tuple(batch_ts), n_passes=N_PASSES)
    return _PROGRAMS[key]


def _host_prep(q, d_pos, d_neg, mask_pos, mask_neg):
    q = np.asarray(q, dtype=np.float32)
    d_pos = np.asarray(d_pos, dtype=np.float32)
    d_neg = np.asarray(d_neg, dtype=np.float32)
    mask_pos = np.asarray(mask_pos)
    mask_neg = np.asarray(mask_neg)

    # Compact: move each doc's valid tokens to the front, pad the tail
    # with copies of token 0 (always valid per setup).  Plain max over
    # the first t_hat columns == masked max, exactly.
    def compact(d, mask):
        out = np.empty_like(d)
        for b in range(d.shape[0]):
            v = d[b, mask[b] != 0]
            out[b, : len(v)] = v
            out[b, len(v) :] = d[b, 0]
        return out

    dp = compact(d_pos, mask_pos)
    dn = compact(d_neg, mask_neg)
    cp = mask_pos.sum(1)
    cn = mask_neg.sum(1)
    # Per core: sort its 16 docs by valid-count descending so doc slots
    # with similar counts share a reduce batch; per-batch T is the max
    # count over the cores' docs in that batch's two slots.
    perms = []
    sorted_counts = np.zeros((NCORES, DOCS_PER_CORE), np.int64)
    for c in range(NCORES):
        counts = np.concatenate([cp[8 * c : 8 * c + 8], cn[8 * c : 8 * c + 8]])
        perm = np.argsort(-counts, kind="stable")
        perms.append(perm)
        sorted_counts[c] = counts[perm]
    n_batches = DOCS_PER_CORE // RED_BATCH
    batch_ts = tuple(
        min(SD, (int(sorted_counts[:, b * RED_BATCH : (b + 1) * RED_BATCH].max()) + 3)
            // 4 * 4)
        for b in range(n_batches)
    )

    def split_hi_lo(x):
        hi = x.astype(np.float16)
        lo = (x - hi.astype(np.float32)).astype(np.float16)
        return hi, lo

    # qT[h, q*SQ + s]
    qT = np.ascontiguousarray(q.transpose(2, 0, 1).reshape(H, B * SQ))
    qh, ql = split_hi_lo(qT)
    # dT[h, doc, t]
    dpT = dp.transpose(2, 0, 1)  # [H, 64, 512]
    dnT = dn.transpose(2, 0, 1)

    ones2 = np.zeros((H, 2), np.float32)
    ones2[:SQ, 0] = 1.0
    ones2[SQ:, 1] = 1.0

    in_maps = []
    for c in range(NCORES):
        dT_c = np.ascontiguousarray(
            np.concatenate(
                [dpT[:, 8 * c : 8 * c + 8, :], dnT[:, 8 * c : 8 * c + 8, :]], axis=1
            )[:, perms[c], :].reshape(H, DOCS_PER_CORE * SD)
        )
        dh_c, dl_c = split_hi_lo(dT_c)
        in_maps.append(
            {
                "qh": qh,
                "ql": ql,
                "dh": dh_c,
                "dl": dl_c,
                "ones2": ones2,
            }
        )
    return in_maps, batch_ts, perms


def _host_epilogue(results, perms):
    # scores_raw rows: partition = pg*16 + d_local; cols: g*2 + j
    # query = 2*(8*g + pg) + j ; doc_local d: 0-7 pos docs 8c+d, 8-15 neg.
    dist = np.zeros((B, 2 * B), np.float32)
    for c in range(NCORES):
        arr = np.asarray(results[c]["scores_raw"])  # [128, 8]
        arr = arr.reshape(8, 16, PAIR_GROUPS, 2)  # [pg, d, g, j]
        s_qd = arr.transpose(2, 0, 3, 1).reshape(B, DOCS_PER_CORE)  # [query, slot]
        inv = np.empty_like(perms[c])
        inv[perms[c]] = np.arange(DOCS_PER_CORE)
        s_qd = s_qd[:, inv]  # [query, original local doc]
        dist[:, 8 * c : 8 * c + 8] = s_qd[:, 0:8]
        dist[:, B + 8 * c : B + 8 * c + 8] = s_qd[:, 8:16]

    d64 = dist.astype(np.float64)
    m = d64.max(axis=1, keepdims=True)
    logz = np.log(np.exp(d64 - m).sum(axis=1)) + m[:, 0]
    lbl = np.arange(B)
    loss = -(d64[lbl, lbl] - logz).mean()
    return np.array(loss, dtype=np.float32)


def kernel(q, d_pos, d_neg, mask_pos, mask_neg):
    in_maps, batch_ts, perms = _host_prep(q, d_pos, d_neg, mask_pos, mask_neg)
    nc = _get_program(batch_ts)
    res = run_bass_kernel_spmd(nc, in_maps, list(range(NCORES)), trace=False)
    return _host_epilogue(res.results, perms)
